# revision 17
# baseline (speedup 1.0000x reference)
"""MeanAggregator (GNN segment-mean) Bass kernel for 8 Trainium2 NeuronCores.

Reference computation:
    gathered = features[edge_dst]                       # [E, D]
    sums     = segment_sum(gathered, edge_seg, B)       # [B, D]
    counts   = segment_sum(ones(E), edge_seg, B)        # [B]
    out      = sums / counts[:, None]                   # [B, D]

Strategy: shard output nodes (segments) contiguously across the 8 cores;
edge_seg is sorted, so each core owns its output rows outright -- no
collectives.

v4 fast path (uniform degree K with a self-loop column):
  * features are converted to bf16 host-side: halves the gather bytes and
    doubles the PE matmul rate; the 2e-2 tolerance leaves bf16's ~2e-3
    mean-of-17 error a wide margin.
  * neighbor rows are fetched with the GpSimd `dma_gather` ucode.  Indices
    are int16 relative to one of four 26624-row windows.  The gather is
    descriptor-rate bound (~35ns per 256B row per SDMA engine, independent
    of HBM locality -- measured via IDXSEQ/IDX0 ablations), so FEWER+larger
    calls win: 4 windows x 9 groups = 36 calls over 4 SWDGE queues (the
    ucode cap; each queue has a dedicated Q7 cpu pair for descgen).
  * the layout is computed from the actual indices at kernel() time: per
    (group of 6 tiles, window) the 8 cores' entries pack contiguously with
    shared per-tile spans (max count across cores, ~8% padding).  Pad
    slots re-read the preceding real row (row-buffer hit; an all-same-row
    pad serializes HBM banks -- IDX0 measured 2.4x slower).  Every slot is
    always written (no NaN hazard, no memsets).  A block straddling two
    tiles is matmul'd by both with its own nid column each.
  * the gathered rows land row-sorted, not per-node; a per-block [128,128]
    bf16 0/1 selection matrix routes them: out[node] += sel.T @ G_block on
    the PE, accumulating in PSUM.  ALL of a tile's sels build in ONE DVE
    tensor_tensor (iota is_equal nid), nid columns broadcast over the node
    axis via stride-0 APs; int16 operands keep DVE 2x mode.  This cut DVE
    busy ~2.6x vs per-sel tensor_scalar: DVE 2x-mode time serializes
    against SWDGE descgen (SBUF port conflict with the descriptor rings),
    so it trades ~1:1 against gather time.  GpSimd must NOT build sels.
  * the self-loop column is a contiguous slab (HWDGE line rate) folded in
    with an identity matmul; the 1/K mean scale is folded into the
    Activation-engine PSUM drain (activation Copy with scale), so sels
    stay exact 0/1.
  * engine budget per core: gather ~225us (the wall), PE ~1050 matmuls
    ~90-170us, DVE ~50us, ACT ~10us -- compute fully hides under the
    gather; total ~= gather + one group's compute tail.

Fallback (v1, arbitrary counts): per-column indirect DMAs + VectorE tree
reduction with per-edge weights.
"""

import sys

for _p in ("/opt/trn_rl_repo", "/root/.axon_site/_ro/trn_rl_repo"):
    if _p not in sys.path:
        sys.path.append(_p)

import numpy as np

from concourse import bacc, bass, mybir
import concourse.tile as tile
from concourse.bass_utils import run_bass_kernel_spmd

TRACE = False            # set by test.py to profile the HW run
TRACE_KWARGS = {"trace": True}
LAST_RESULT = None

P = 128          # SBUF partitions = nodes per tile
D = 128          # feature dim
N_CORES = 8
N_TOTAL = 100000  # feature table rows

WINDOW = 26624           # rows per int16-index window (26624 = 208*128; 4
                         # windows minimizes dma_gather calls and padding --
                         # HW-tuned; int16 idx caps a window at 32767 rows)
N_WINDOWS = 4            # ceil(N_TOTAL / WINDOW)


def set_window(w):
    global WINDOW, N_WINDOWS
    WINDOW = w
    N_WINDOWS = -(-N_TOTAL // w)

G_BUFS = 2               # gather-buffer double/triple buffering
GROUP_TILES = 6          # tiles per gather group
MAX_CALL_BLOCKS = 42     # max 128-row blocks per dma_gather call
V3_QUEUES = 4            # SWDGE queues to round-robin gather calls over
SINGLE_PACKET = False    # True packs a call into one packet but caps it at
                         # 64 descs = 1024 idxs; False allows larger calls
SEL_PATTERN = "vvvvva"   # unused in v4 (kept for test.py compat)
BATCH_SEL = True         # one tensor_tensor builds ALL of a tile's sels
                         # (broadcast nid via stride-0 AP); False falls back
                         # to one tensor_scalar per sel on DVE
SEL_BUFS = 4             # sel tile lookahead (per-tile batches)
PRELOAD = True           # load all meta + self-loop slabs once up front
ALIGN_TILES = False      # True: block-align each tile's span (fewer matmuls,
                         # more gather rows); False: pack tight (fewer rows,
                         # boundary blocks matmul'd by both tiles)


def _scratch_size():
    """SWDGE ring must hold one call's descriptors (16 B/desc mirror of
    ucode); the ring is a power-of-two circular buffer."""
    # single_packet ucode packs 16 idxs/descriptor -> MCB*8 descs + slack
    need = 16 * (MAX_CALL_BLOCKS * 8 + 64)
    sz = 16384
    while sz < need:
        sz *= 2
    return sz


def _bf16():
    return mybir.dt.np(mybir.dt.bfloat16)


# --------------------------------------------------------------------------
# v3 host-side planning


class _V3Plan:
    """Static per-core program layout, derived from the actual indices."""

    __slots__ = ("groups", "meta", "tbg_max", "meta_max", "n_tiles", "scale",
                 "njmax")

    def __init__(self):
        self.groups = []


def build_program_v3(plan, repeats: int = 1, parts: str = "all") -> bass.Bass:
    """Bass program run identically on every core.

    Inputs per core:
      features [N_TOTAL, D] bf16  (replicated table)
      selfloop [n_tiles*P, D] bf16 (this core's self-loop rows, contiguous)
      meta     [128, meta_total] i16 (idx words + nid columns per group)
    Output per core:
      out      [n_tiles*P, D] f32

    parts: "all" (normal), "gather" (skip compute; timing ablation only),
    "compute" (skip gathers; broken on purpose, timing ablation only).
    """
    n_tiles = plan.n_tiles
    nodes = n_tiles * P
    bf16 = mybir.dt.bfloat16
    nc = bacc.Bacc("TRN2", target_bir_lowering=False,
                   num_swdge_queues=V3_QUEUES,
                   dynamic_dma_scratch_size=_scratch_size())
    feat = nc.declare_dram_parameter("features", [N_TOTAL, D],
                                     bf16, isOutput=False)
    sl = nc.declare_dram_parameter("selfloop", [nodes, D],
                                   bf16, isOutput=False)
    meta = nc.declare_dram_parameter("meta", [128, plan.meta.shape[1]],
                                     mybir.dt.int16, isOutput=False)
    out = nc.declare_dram_parameter("out", [nodes, D],
                                    mybir.dt.float32, isOutput=True)
    gt_max = max(g["gt"] for g in plan.groups)

    njmax = max(1, plan.njmax)
    with tile.TileContext(nc) as tc:
        with tc.tile_pool(name="const", bufs=1) as cp, \
             tc.tile_pool(name="meta", bufs=3) as mp, \
             tc.tile_pool(name="slab", bufs=3) as slp, \
             tc.tile_pool(name="gath", bufs=G_BUFS) as gp, \
             tc.tile_pool(name="sel", bufs=SEL_BUFS) as sp_, \
             tc.tile_pool(name="ps", bufs=4, space="PSUM") as pp, \
             tc.tile_pool(name="res", bufs=3) as rp:
            # iota_row[p, q] = q ; piota[p, q] = p      (int16)
            iota_row = cp.tile([P, P], mybir.dt.int16)
            nc.gpsimd.iota(iota_row[:], pattern=[[1, P]], channel_multiplier=0)
            piota_i = cp.tile([P, P], mybir.dt.int32)
            nc.gpsimd.iota(piota_i[:], pattern=[[0, P]],
                           channel_multiplier=1)
            piota = cp.tile([P, P], mybir.dt.float32)
            nc.vector.tensor_copy(out=piota[:], in_=piota_i[:])
            # iota_b[p, (j, q)] = q  (int16): batched sel-build operand
            iota_b = cp.tile([P, njmax * P], mybir.dt.int16)
            nc.gpsimd.iota(iota_b[:], pattern=[[0, njmax], [1, P]],
                           channel_multiplier=0)
            # I[p, q] = (q == p)                        (bf16; 1/K folded
            # into the PSUM drain activation)
            i_eye = cp.tile([P, P], bf16)
            nc.vector.tensor_scalar(
                out=i_eye[:], in0=iota_row[:],
                scalar1=piota[:, 0:1], scalar2=1.0,
                op0=mybir.AluOpType.is_equal, op1=mybir.AluOpType.mult)
            if PRELOAD:
                # load ALL meta + self-loop slabs once up front: removes
                # per-group HWDGE input DMAs from the steady state (they
                # contend with the gather rings on the SDMA engines)
                meta_all = cp.tile([128, plan.meta.shape[1]], mybir.dt.int16)
                nc.sync.dma_start(out=meta_all[:], in_=meta[:, :])
                slab_all = cp.tile([P, n_tiles * D], bf16)
                nc.scalar.dma_start(
                    out=slab_all[:].rearrange("p (b d) -> p b d", d=D),
                    in_=sl[:, :].rearrange("(b p) d -> p b d", p=P))
            for rep in range(repeats):
                for g in plan.groups:
                    gt, tbg = g["gt"], g["tbg"]
                    if PRELOAD:
                        meta_t, moff = meta_all, g["goff"]
                        slab_t, soff = slab_all, (g["r0"] // P) * D
                    else:
                        moff = soff = 0
                        meta_t = mp.tile([128, plan.meta_max],
                                         mybir.dt.int16, tag="meta")
                        nc.sync.dma_start(
                            out=meta_t[:, :g["glen"]],
                            in_=meta[:, g["goff"]:g["goff"] + g["glen"]])
                        slab_t = slp.tile([P, gt_max * D], bf16, tag="slab")
                        nc.scalar.dma_start(
                            out=slab_t[:, :gt * D].rearrange(
                                "p (b d) -> p b d", d=D),
                            in_=sl[g["r0"]:g["r0"] + gt * P, :].rearrange(
                                "(b p) d -> p b d", p=P))
                    G = gp.tile([P, plan.tbg_max * D], bf16, tag="g")
                    if parts == "compute":
                        g_calls = []
                        # keep compute numerics finite without gathers
                        nc.vector.memset(G[:], 0.0)
                    else:
                        g_calls = g["calls"]
                    gi = g["r0"] // (P * GROUP_TILES)
                    for c, (w, b0, nb, iw0) in enumerate(g_calls):
                        wbase = w * WINDOW
                        wsize = min(WINDOW, N_TOTAL - wbase)
                        nc.gpsimd.dma_gather(
                            out_ap=G[:, b0 * D:(b0 + nb) * D].rearrange(
                                "p (b d) -> p b d", d=D),
                            in_ap=feat[wbase:wbase + wsize],
                            idxs_ap=meta_t[:, moff + iw0:moff + iw0 + nb * 8],
                            num_idxs=nb * P,
                            num_idxs_reg=nb * P,
                            elem_size=D,
                            single_packet=SINGLE_PACKET,
                            queue_num=(c + gi) % V3_QUEUES,
                        )
                    obuf = rp.tile([P, gt_max * D], mybir.dt.float32,
                                   tag="o")
                    if parts == "gather":
                        # touch G so the gather isn't dead-code'd, then skip
                        # the compute stage
                        dummy = rp.tile([P, D], mybir.dt.float32, tag="dm")
                        nc.vector.tensor_copy(out=dummy[:],
                                              in_=G[:, :2 * D].bitcast(
                                                  mybir.dt.float32))
                        nc.sync.dma_start(
                            out=out[g["r0"]:g["r0"] + P, :], in_=dummy[:])
                        continue
                    for (ti, mm, c0) in g["tiles"]:
                        nj = len(mm)
                        sel_t = sp_.tile([P, njmax * P], bf16, tag="sel")
                        if nj:
                            if BATCH_SEL:
                                # sel_t[p, (j, q)] = (q == nid_j[p]) in ONE
                                # DVE op: nid column j broadcast over q via
                                # stride-0 AP
                                iota3 = iota_b[:, :nj * P].rearrange(
                                    "p (n q) -> p n q", q=P)
                                nid3 = meta_t[:, moff + c0:moff + c0 + nj].rearrange(
                                    "p (n o) -> p n o", o=1)
                                nid_bc, iota_bc = bass.broadcast_tensor_aps(
                                    nid3, iota3)
                                nc.vector.tensor_tensor(
                                    out=sel_t[:, :nj * P].rearrange(
                                        "p (n q) -> p n q", q=P),
                                    in0=iota_bc, in1=nid_bc,
                                    op=mybir.AluOpType.is_equal)
                            else:
                                for j in range(nj):
                                    nc.vector.tensor_scalar(
                                        out=sel_t[:, j * P:(j + 1) * P],
                                        in0=iota_row[:],
                                        scalar1=meta_t[:, moff + c0 + j:moff + c0 + j + 1],
                                        scalar2=1.0,
                                        op0=mybir.AluOpType.is_equal,
                                        op1=mybir.AluOpType.mult)
                        ps = pp.tile([P, D], mybir.dt.float32, tag="ps")
                        nc.tensor.matmul(
                            out=ps[:], lhsT=i_eye[:],
                            rhs=slab_t[:, soff + ti * D:soff + (ti + 1) * D],
                            start=True, stop=(nj == 0))
                        for j, gblk in enumerate(mm):
                            nc.tensor.matmul(
                                out=ps[:], lhsT=sel_t[:, j * P:(j + 1) * P],
                                rhs=G[:, gblk * D:(gblk + 1) * D],
                                start=False, stop=(j == nj - 1))
                        # PSUM holds K*mean; scale by 1/K on the drain
                        nc.scalar.activation(
                            out=obuf[:, ti * D:(ti + 1) * D], in_=ps[:],
                            func=mybir.ActivationFunctionType.Copy,
                            scale=float(plan.scale))
                    nc.sync.dma_start(
                        out=out[g["r0"]:g["r0"] + gt * P, :].rearrange(
                            "(b p) d -> p b d", p=P),
                        in_=obuf[:, :gt * D].rearrange(
                            "p (b d) -> p b d", d=D))
    nc.compile()
    return nc


# --------------------------------------------------------------------------
# v1 fallback: arbitrary sorted-or-not edge_seg


def build_program(n_tiles: int, K: int, weighted: bool,
                  g_bufs: int = 3, repeats: int = 1) -> bass.Bass:
    """Fallback program (per-column indirect DMA + tree reduction)."""
    nodes = n_tiles * P
    nc = bacc.Bacc("TRN2", target_bir_lowering=False)
    feat = nc.declare_dram_parameter("features", [N_TOTAL, D],
                                     mybir.dt.float32, isOutput=False)
    idx = nc.declare_dram_parameter("idx", [nodes, K],
                                    mybir.dt.int32, isOutput=False)
    if weighted:
        wts = nc.declare_dram_parameter("wts", [nodes, K],
                                        mybir.dt.float32, isOutput=False)
    out = nc.declare_dram_parameter("out", [nodes, D],
                                    mybir.dt.float32, isOutput=True)

    with tile.TileContext(nc) as tc:
        with tc.tile_pool(name="gath", bufs=g_bufs) as gp, \
             tc.tile_pool(name="io", bufs=4) as iop, \
             tc.tile_pool(name="res", bufs=4) as rp:
            for t in range(n_tiles * repeats):
                t = t % n_tiles
                sl = slice(t * P, (t + 1) * P)
                idx_t = iop.tile([P, K], mybir.dt.int32, tag="idx")
                nc.sync.dma_start(out=idx_t[:], in_=idx[sl, :])
                G = gp.tile([P, K * D], mybir.dt.float32, tag="g")
                for j in range(K):
                    nc.gpsimd.indirect_dma_start(
                        out=G[:, j * D:(j + 1) * D],
                        out_offset=None,
                        in_=feat[:],
                        in_offset=bass.IndirectOffsetOnAxis(
                            ap=idx_t[:, j:j + 1], axis=0),
                    )
                if weighted:
                    w_t = iop.tile([P, K], mybir.dt.float32, tag="w")
                    nc.sync.dma_start(out=w_t[:], in_=wts[sl, :])
                    for j in range(K):
                        nc.vector.tensor_scalar_mul(
                            out=G[:, j * D:(j + 1) * D],
                            in0=G[:, j * D:(j + 1) * D],
                            scalar1=w_t[:, j:j + 1],
                        )
                cur = K
                while cur > 1:
                    h = cur // 2
                    nc.vector.tensor_tensor(
                        out=G[:, :h * D],
                        in0=G[:, :h * D],
                        in1=G[:, h * D:2 * h * D],
                        op=mybir.AluOpType.add,
                    )
                    if cur % 2:
                        nc.vector.tensor_tensor(
                            out=G[:, (h - 1) * D:h * D],
                            in0=G[:, (h - 1) * D:h * D],
                            in1=G[:, (cur - 1) * D:cur * D],
                            op=mybir.AluOpType.add,
                        )
                    cur = h
                o_t = rp.tile([P, D], mybir.dt.float32, tag="o")
                if weighted:
                    nc.vector.tensor_copy(out=o_t[:], in_=G[:, :D])
                else:
                    nc.vector.tensor_scalar_mul(out=o_t[:], in0=G[:, :D],
                                                scalar1=1.0 / K)
                nc.sync.dma_start(out=out[sl, :], in_=o_t[:])
    nc.compile()
    return nc


def _prep_general(edge_seg, edge_dst, B):
    E = edge_dst.shape[0]
    order = np.argsort(edge_seg, kind="stable")
    sseg = edge_seg[order].astype(np.int64)
    sdst = edge_dst[order].astype(np.int32)
    counts = np.bincount(sseg, minlength=B).astype(np.int64)
    K = max(int(counts.max()), 1) if E else 1
    starts = np.zeros(B, np.int64)
    np.cumsum(counts[:-1], out=starts[1:])
    pos = np.arange(E, dtype=np.int64) - np.repeat(starts, counts)
    idx_mat = np.zeros((B, K), np.int32)
    wts_mat = np.zeros((B, K), np.float32)
    idx_mat[sseg, pos] = sdst
    inv = np.zeros(B, np.float32)
    nz = counts > 0
    inv[nz] = 1.0 / counts[nz]
    wts_mat[sseg, pos] = inv[sseg]
    return idx_mat, wts_mat, K


def kernel(features, edge_seg, edge_dst, num_nodes=None, **_unused):
    features = np.ascontiguousarray(np.asarray(features, dtype=np.float32))
    edge_seg = np.asarray(edge_seg)
    edge_dst = np.asarray(edge_dst)
    E = int(edge_dst.shape[0])
    if num_nodes is not None:
        B = int(np.asarray(num_nodes))
    else:
        B = int(edge_seg.max()) + 1

    K = E // B if B and E % B == 0 else 0
    uniform = K > 0 and np.array_equal(
        edge_seg, np.repeat(np.arange(B, dtype=edge_seg.dtype), K))
    if uniform:
        idx_mat = np.ascontiguousarray(edge_dst.reshape(B, K).astype(np.int32))
        wts_mat = None
    else:
        idx_mat, wts_mat, K = _prep_general(edge_seg, edge_dst, B)

    npc = -(-B // N_CORES)           # nodes per core (ceil)
    n_tiles = -(-npc // P)
    nodes_pad = n_tiles * P
    weighted = wts_mat is not None

    in_maps = None
    if not weighted:
        try:
            features_bf = features.astype(_bf16())
            in_maps, plans = prep_v3_core_maps(features_bf, idx_mat, B,
                                               npc, n_tiles)
            nc = build_program_v3(plans)
            for m in in_maps:
                m.pop("_plan", None)
        except (ValueError, OverflowError, AssertionError):
            in_maps = None

    if in_maps is None:
        nc = build_program(n_tiles, K, weighted)
        in_maps = []
        for c in range(N_CORES):
            lo = c * npc
            hi = min(B, (c + 1) * npc)
            idx_c = np.zeros((nodes_pad, K), np.int32)
            if hi > lo:
                idx_c[:hi - lo] = idx_mat[lo:hi]
            m = {"features": features, "idx": idx_c}
            if weighted:
                w_c = np.zeros((nodes_pad, K), np.float32)
                if hi > lo:
                    w_c[:hi - lo] = wts_mat[lo:hi]
                m["wts"] = w_c
            in_maps.append(m)

    kw = dict(TRACE_KWARGS) if TRACE else {}
    res = run_bass_kernel_spmd(nc, in_maps, list(range(N_CORES)), **kw)
    global LAST_RESULT
    LAST_RESULT = res
    parts = []
    for c in range(N_CORES):
        lo = c * npc
        hi = min(B, (c + 1) * npc)
        if hi > lo:
            parts.append(res.results[c]["out"][:hi - lo])
    return np.concatenate(parts, axis=0)


def prep_v3_core_maps(features_bf, idx_mat, B, npc, n_tiles):
    """Per-core host prep.  The program layout is data-dependent and differs
    per core, but SPMD needs ONE program: merge the per-core plans into a
    shared super-layout (max blocks per (group, window, tile) across cores)
    and re-emit each core's meta in that layout."""
    K = idx_mat.shape[1]
    if not np.array_equal(idx_mat[:, 0], np.arange(B, dtype=idx_mat.dtype)):
        raise ValueError("v3 requires a self-loop first column")
    Kg = K - 1
    nodes_pad = n_tiles * P
    bf = _bf16()
    wbounds = [w * WINDOW for w in range(N_WINDOWS + 1)]
    wbounds[-1] = N_TOTAL + 1

    # pass 1: per (core, tile, window) entries + exact block counts
    core_ent = []
    counts = np.zeros((N_CORES, n_tiles, N_WINDOWS), np.int64)
    sls = []
    for c in range(N_CORES):
        lo = c * npc
        hi = min(B, (c + 1) * npc)
        cols = np.full((nodes_pad, Kg), -1, np.int32)
        sl_c = np.zeros((nodes_pad, D), bf)
        if hi > lo:
            cols[:hi - lo] = idx_mat[lo:hi, 1:]
            n_sl = min(nodes_pad, N_TOTAL - lo)
            sl_c[:n_sl] = features_bf[lo:lo + n_sl]
        sls.append(sl_c)
        ent = {}
        for t in range(n_tiles):
            rf = cols[t * P:(t + 1) * P]
            r = rf.ravel().astype(np.int64)
            p = np.repeat(np.arange(P, dtype=np.int64), Kg)
            keep = r >= 0
            r, p = r[keep], p[keep]
            o = np.argsort(r, kind="stable")
            rs, ps = r[o], p[o]
            bounds = np.searchsorted(rs, wbounds)
            for w in range(N_WINDOWS):
                s0, s1 = bounds[w], bounds[w + 1]
                ent[(t, w)] = (rs[s0:s1] - w * WINDOW, ps[s0:s1])
                counts[c, t, w] = s1 - s0
        core_ent.append(ent)
    # shared layout: per-(tile, window) slot spans = max count across
    # cores.  Tiles pack contiguously inside each (group, window) region
    # (no per-tile block ceiling); a block straddling two tiles is simply
    # matmul'd by both with its own nid column each (entries of the other
    # tile carry nid -1 there).
    scnt = counts.max(axis=0)         # [n_tiles, N_WINDOWS] shared spans

    plan = _V3Plan()
    plan.n_tiles = n_tiles
    plan.scale = 1.0 / K
    metas = [[] for _ in range(N_CORES)]
    goff = 0
    tbg_max = 0
    meta_max = 0
    for g0 in range(0, n_tiles, GROUP_TILES):
        tiles = list(range(g0, min(g0 + GROUP_TILES, n_tiles)))
        gt = len(tiles)
        s0 = {}                       # (t, w) -> start slot in its region
        rblocks = []                  # blocks per window region
        for w in range(N_WINDOWS):
            s = 0
            for t in tiles:
                s0[(t, w)] = s
                span = int(scnt[t, w])
                if ALIGN_TILES:
                    span = -(-span // P) * P
                s += span
            rblocks.append(-(-s // P))
        breg = np.cumsum([0] + rblocks)   # block offset of each region
        tbg = int(breg[-1])
        tbg_max = max(tbg_max, tbg)
        # calls: split each window region at MAX_CALL_BLOCKS
        call_list = []                # (w, b0, nb, iw0)
        iw = 0
        for w in range(N_WINDOWS):
            cb = 0
            while cb < rblocks[w]:
                nb = min(MAX_CALL_BLOCKS, rblocks[w] - cb)
                call_list.append((w, int(breg[w]) + cb, nb, iw))
                iw += nb * 8
                cb += nb
        # matmul lists: tile t covers blocks [s0//P, ceil((s0+span)/P))
        tile_list = []
        col = 0
        nidcol_ix = {}
        for ti, t in enumerate(tiles):
            mm = []
            c0 = iw + col
            for w in range(N_WINDOWS):
                span = int(scnt[t, w])
                if not span:
                    continue
                j0 = s0[(t, w)] // P
                j1 = -(-(s0[(t, w)] + span) // P)
                for j in range(j0, j1):
                    mm.append(int(breg[w]) + j)
                    nidcol_ix[(t, w, j)] = col
                    col += 1
            tile_list.append((ti, mm, c0))
        ncols = col
        glen = iw + ncols
        # per-core meta piece
        for c in range(N_CORES):
            ent = core_ent[c]
            piece = np.zeros((128, glen), np.int16)
            for w in range(N_WINDOWS):
                idxarr = np.zeros(rblocks[w] * P, np.int16)
                nidarr = np.full(rblocks[w] * P, -1, np.int16)
                for t in tiles:
                    rel, ps = ent[(t, w)]
                    a = s0[(t, w)]
                    span = int(scnt[t, w])
                    if ALIGN_TILES:
                        span = -(-span // P) * P
                    idxarr[a:a + rel.shape[0]] = rel.astype(np.int16)
                    # pad slots re-read the last real row (row-buffer hit)
                    # instead of hammering row 0 of the window
                    if span > rel.shape[0]:
                        pad_idx = rel[-1] if rel.shape[0] else 0
                        idxarr[a + rel.shape[0]:a + span] = pad_idx
                    if a + span < idxarr.shape[0] and t == tiles[-1]:
                        # region tail (block rounding) after the last tile
                        idxarr[a + span:] = idxarr[a + span - 1] if span \
                            else 0
                    nidarr[a:a + ps.shape[0]] = ps.astype(np.int16)
                for (w2, b0, nb, iw0) in call_list:
                    if w2 != w:
                        continue
                    cb0 = b0 - int(breg[w])
                    e = idxarr[cb0 * P:(cb0 + nb) * P]
                    arr = e.reshape(nb * 8, 16).T
                    piece[:, iw0:iw0 + nb * 8] = np.tile(arr, (8, 1))
                # nid columns for this window, per tile (int16, one per sel)
                for ti, t in enumerate(tiles):
                    span = int(scnt[t, w])
                    if not span:
                        continue
                    a, b = s0[(t, w)], s0[(t, w)] + span
                    cnt_c = ent[(t, w)][1].shape[0]
                    j0 = a // P
                    j1 = -(-b // P)
                    for j in range(j0, j1):
                        colv = np.full(P, -1, np.int16)
                        lo_s = max(a, j * P)
                        hi_s = min(a + cnt_c, (j + 1) * P)
                        if hi_s > lo_s:
                            colv[lo_s - j * P:hi_s - j * P] = \
                                nidarr[lo_s:hi_s]
                        cix = nidcol_ix[(t, w, j)]
                        piece[:, iw + cix] = colv
            metas[c].append(piece)
        plan.groups.append(dict(r0=g0 * P, gt=gt, goff=goff, glen=glen,
                                tbg=tbg, calls=call_list, tiles=tile_list))
        goff += glen
        meta_max = max(meta_max, glen)
    plan.tbg_max = tbg_max
    plan.meta_max = meta_max
    plan.njmax = max(len(mm) for g in plan.groups for (_, mm, _) in g["tiles"])
    in_maps = []
    for c in range(N_CORES):
        meta_c = np.ascontiguousarray(np.concatenate(metas[c], axis=1))
        in_maps.append({"features": features_bf, "selfloop": sls[c],
                        "meta": meta_c})
    plan.meta = in_maps[0]["meta"]
    return in_maps, plan



# revision 19
# speedup vs baseline: 1.2366x; 1.2366x over previous
"""MeanAggregator (GNN segment-mean) Bass kernel for 8 Trainium2 NeuronCores.

Reference computation:
    gathered = features[edge_dst]                       # [E, D]
    sums     = segment_sum(gathered, edge_seg, B)       # [B, D]
    counts   = segment_sum(ones(E), edge_seg, B)        # [B]
    out      = sums / counts[:, None]                   # [B, D]

Strategy: shard output nodes (segments) contiguously across the 8 cores;
edge_seg is sorted, so each core owns its output rows outright -- no
collectives.

v4 fast path (uniform degree K with a self-loop column):
  * features are converted to bf16 host-side: halves the gather bytes and
    doubles the PE matmul rate; the 2e-2 tolerance leaves bf16's ~2e-3
    mean-of-17 error a wide margin.
  * neighbor rows are fetched with the GpSimd `dma_gather` ucode.  Indices
    are int16 relative to one of four 26624-row windows.  The gather is
    descriptor-rate bound (~35ns per 256B row per SDMA engine, independent
    of HBM locality -- measured via IDXSEQ/IDX0 ablations), so FEWER+larger
    calls win: 4 windows x 9 groups = 36 calls over 4 SWDGE queues (the
    ucode cap; each queue has a dedicated Q7 cpu pair for descgen).
  * the layout is computed from the actual indices at kernel() time: per
    (group of 6 tiles, window) the 8 cores' entries pack contiguously with
    shared per-tile spans (max count across cores, ~8% padding).  Pad
    slots re-read the preceding real row (row-buffer hit; an all-same-row
    pad serializes HBM banks -- IDX0 measured 2.4x slower).  Every slot is
    always written (no NaN hazard, no memsets).  A block straddling two
    tiles is matmul'd by both with its own nid column each.
  * the gathered rows land row-sorted, not per-node; a per-block [128,128]
    bf16 0/1 selection matrix routes them: out[node] += sel.T @ G_block on
    the PE, accumulating in PSUM.  ALL of a tile's sels build in ONE DVE
    tensor_tensor (iota is_equal nid), nid columns broadcast over the node
    axis via stride-0 APs; int16 operands keep DVE 2x mode.  This cut DVE
    busy ~2.6x vs per-sel tensor_scalar: DVE 2x-mode time serializes
    against SWDGE descgen (SBUF port conflict with the descriptor rings),
    so it trades ~1:1 against gather time.  GpSimd must NOT build sels.
  * the self-loop column is a contiguous slab (HWDGE line rate) folded in
    with an identity matmul; the 1/K mean scale is folded into the
    Activation-engine PSUM drain (activation Copy with scale), so sels
    stay exact 0/1.
  * all meta + self-loop slabs preload once at start (PRELOAD): the steady
    state has no HWDGE input DMAs contending with the gather rings; G is
    triple-buffered (G_BUFS=3) so all 4 queues stay fed across group
    boundaries.
  * engine budget per core: gather ~225us (the wall), PE ~1050 matmuls
    ~90-170us, DVE ~50us, ACT ~10us -- compute fully hides under the
    gather; total ~= gather + one group's compute tail.  Measured total
    ~225-240us (vs 339us baseline; shared-host timing variance is large --
    identical programs measured 1.5x apart minutes apart).

Fallback (v1, arbitrary counts): per-column indirect DMAs + VectorE tree
reduction with per-edge weights.
"""

import sys

for _p in ("/opt/trn_rl_repo", "/root/.axon_site/_ro/trn_rl_repo"):
    if _p not in sys.path:
        sys.path.append(_p)

import numpy as np

from concourse import bacc, bass, mybir
import concourse.tile as tile
from concourse.bass_utils import run_bass_kernel_spmd

TRACE = False            # set by test.py to profile the HW run
TRACE_KWARGS = {"trace": True}
LAST_RESULT = None

P = 128          # SBUF partitions = nodes per tile
D = 128          # feature dim
N_CORES = 8
N_TOTAL = 100000  # feature table rows

WINDOW = 26624           # rows per int16-index window (26624 = 208*128; 4
                         # windows minimizes dma_gather calls and padding --
                         # HW-tuned; int16 idx caps a window at 32767 rows)
N_WINDOWS = 4            # ceil(N_TOTAL / WINDOW)


def set_window(w):
    global WINDOW, N_WINDOWS
    WINDOW = w
    N_WINDOWS = -(-N_TOTAL // w)

G_BUFS = 3               # gather-buffer pipelining depth (3 keeps all 4
                         # SWDGE queues fed across group boundaries)
GROUP_TILES = 6          # tiles per gather group
MAX_CALL_BLOCKS = 42     # max 128-row blocks per dma_gather call
V3_QUEUES = 4            # SWDGE queues to round-robin gather calls over
SINGLE_PACKET = False    # True packs a call into one packet but caps it at
                         # 64 descs = 1024 idxs; False allows larger calls
SEL_PATTERN = "vvvvva"   # unused in v4 (kept for test.py compat)
BATCH_SEL = True         # one tensor_tensor builds ALL of a tile's sels
                         # (broadcast nid via stride-0 AP); False falls back
                         # to one tensor_scalar per sel on DVE
SEL_BUFS = 4             # sel tile lookahead (per-tile batches)
PRELOAD = True           # load all meta + self-loop slabs once up front
ALIGN_TILES = False      # True: block-align each tile's span (fewer matmuls,
                         # more gather rows); False: pack tight (fewer rows,
                         # boundary blocks matmul'd by both tiles)


def _scratch_size():
    """SWDGE ring must hold one call's descriptors (16 B/desc mirror of
    ucode); the ring is a power-of-two circular buffer."""
    # single_packet ucode packs 16 idxs/descriptor -> MCB*8 descs + slack
    need = 16 * (MAX_CALL_BLOCKS * 8 + 64)
    sz = 16384
    while sz < need:
        sz *= 2
    return sz


def _bf16():
    return mybir.dt.np(mybir.dt.bfloat16)


# --------------------------------------------------------------------------
# v3 host-side planning


class _V3Plan:
    """Static per-core program layout, derived from the actual indices."""

    __slots__ = ("groups", "meta", "tbg_max", "meta_max", "n_tiles", "scale",
                 "njmax")

    def __init__(self):
        self.groups = []


def build_program_v3(plan, repeats: int = 1, parts: str = "all") -> bass.Bass:
    """Bass program run identically on every core.

    Inputs per core:
      features [N_TOTAL, D] bf16  (replicated table)
      selfloop [n_tiles*P, D] bf16 (this core's self-loop rows, contiguous)
      meta     [128, meta_total] i16 (idx words + nid columns per group)
    Output per core:
      out      [n_tiles*P, D] f32

    parts: "all" (normal), "gather" (skip compute; timing ablation only),
    "compute" (skip gathers; broken on purpose, timing ablation only).
    """
    n_tiles = plan.n_tiles
    nodes = n_tiles * P
    bf16 = mybir.dt.bfloat16
    nc = bacc.Bacc("TRN2", target_bir_lowering=False,
                   num_swdge_queues=V3_QUEUES,
                   dynamic_dma_scratch_size=_scratch_size())
    feat = nc.declare_dram_parameter("features", [N_TOTAL, D],
                                     bf16, isOutput=False)
    sl = nc.declare_dram_parameter("selfloop", [nodes, D],
                                   bf16, isOutput=False)
    meta = nc.declare_dram_parameter("meta", [128, plan.meta.shape[1]],
                                     mybir.dt.int16, isOutput=False)
    out = nc.declare_dram_parameter("out", [nodes, D],
                                    mybir.dt.float32, isOutput=True)
    gt_max = max(g["gt"] for g in plan.groups)

    njmax = max(1, plan.njmax)
    with tile.TileContext(nc) as tc:
        with tc.tile_pool(name="const", bufs=1) as cp, \
             tc.tile_pool(name="meta", bufs=3) as mp, \
             tc.tile_pool(name="slab", bufs=3) as slp, \
             tc.tile_pool(name="gath", bufs=G_BUFS) as gp, \
             tc.tile_pool(name="sel", bufs=SEL_BUFS) as sp_, \
             tc.tile_pool(name="ps", bufs=4, space="PSUM") as pp, \
             tc.tile_pool(name="res", bufs=3) as rp:
            # iota_row[p, q] = q ; piota[p, q] = p      (int16)
            iota_row = cp.tile([P, P], mybir.dt.int16)
            nc.gpsimd.iota(iota_row[:], pattern=[[1, P]], channel_multiplier=0)
            piota_i = cp.tile([P, P], mybir.dt.int32)
            nc.gpsimd.iota(piota_i[:], pattern=[[0, P]],
                           channel_multiplier=1)
            piota = cp.tile([P, P], mybir.dt.float32)
            nc.vector.tensor_copy(out=piota[:], in_=piota_i[:])
            # iota_b[p, (j, q)] = q  (int16): batched sel-build operand
            iota_b = cp.tile([P, njmax * P], mybir.dt.int16)
            nc.gpsimd.iota(iota_b[:], pattern=[[0, njmax], [1, P]],
                           channel_multiplier=0)
            # I[p, q] = (q == p)                        (bf16; 1/K folded
            # into the PSUM drain activation)
            i_eye = cp.tile([P, P], bf16)
            nc.vector.tensor_scalar(
                out=i_eye[:], in0=iota_row[:],
                scalar1=piota[:, 0:1], scalar2=1.0,
                op0=mybir.AluOpType.is_equal, op1=mybir.AluOpType.mult)
            if PRELOAD:
                # load ALL meta + self-loop slabs once up front: removes
                # per-group HWDGE input DMAs from the steady state (they
                # contend with the gather rings on the SDMA engines)
                meta_all = cp.tile([128, plan.meta.shape[1]], mybir.dt.int16)
                nc.sync.dma_start(out=meta_all[:], in_=meta[:, :])
                slab_all = cp.tile([P, n_tiles * D], bf16)
                nc.scalar.dma_start(
                    out=slab_all[:].rearrange("p (b d) -> p b d", d=D),
                    in_=sl[:, :].rearrange("(b p) d -> p b d", p=P))
            for rep in range(repeats):
                for g in plan.groups:
                    gt, tbg = g["gt"], g["tbg"]
                    if PRELOAD:
                        meta_t, moff = meta_all, g["goff"]
                        slab_t, soff = slab_all, (g["r0"] // P) * D
                    else:
                        moff = soff = 0
                        meta_t = mp.tile([128, plan.meta_max],
                                         mybir.dt.int16, tag="meta")
                        nc.sync.dma_start(
                            out=meta_t[:, :g["glen"]],
                            in_=meta[:, g["goff"]:g["goff"] + g["glen"]])
                        slab_t = slp.tile([P, gt_max * D], bf16, tag="slab")
                        nc.scalar.dma_start(
                            out=slab_t[:, :gt * D].rearrange(
                                "p (b d) -> p b d", d=D),
                            in_=sl[g["r0"]:g["r0"] + gt * P, :].rearrange(
                                "(b p) d -> p b d", p=P))
                    G = gp.tile([P, plan.tbg_max * D], bf16, tag="g")
                    if parts == "compute":
                        g_calls = []
                        # keep compute numerics finite without gathers
                        nc.vector.memset(G[:], 0.0)
                    else:
                        g_calls = g["calls"]
                    gi = g["r0"] // (P * GROUP_TILES)
                    for c, (w, b0, nb, iw0) in enumerate(g_calls):
                        wbase = w * WINDOW
                        wsize = min(WINDOW, N_TOTAL - wbase)
                        nc.gpsimd.dma_gather(
                            out_ap=G[:, b0 * D:(b0 + nb) * D].rearrange(
                                "p (b d) -> p b d", d=D),
                            in_ap=feat[wbase:wbase + wsize],
                            idxs_ap=meta_t[:, moff + iw0:moff + iw0 + nb * 8],
                            num_idxs=nb * P,
                            num_idxs_reg=nb * P,
                            elem_size=D,
                            single_packet=SINGLE_PACKET,
                            queue_num=(c + gi) % V3_QUEUES,
                        )
                    obuf = rp.tile([P, gt_max * D], mybir.dt.float32,
                                   tag="o")
                    if parts == "gather":
                        # touch G so the gather isn't dead-code'd, then skip
                        # the compute stage
                        dummy = rp.tile([P, D], mybir.dt.float32, tag="dm")
                        nc.vector.tensor_copy(out=dummy[:],
                                              in_=G[:, :2 * D].bitcast(
                                                  mybir.dt.float32))
                        nc.sync.dma_start(
                            out=out[g["r0"]:g["r0"] + P, :], in_=dummy[:])
                        continue
                    for (ti, mm, c0) in g["tiles"]:
                        nj = len(mm)
                        sel_t = sp_.tile([P, njmax * P], bf16, tag="sel")
                        if nj:
                            if BATCH_SEL:
                                # sel_t[p, (j, q)] = (q == nid_j[p]) in ONE
                                # DVE op: nid column j broadcast over q via
                                # stride-0 AP
                                iota3 = iota_b[:, :nj * P].rearrange(
                                    "p (n q) -> p n q", q=P)
                                nid3 = meta_t[:, moff + c0:moff + c0 + nj].rearrange(
                                    "p (n o) -> p n o", o=1)
                                nid_bc, iota_bc = bass.broadcast_tensor_aps(
                                    nid3, iota3)
                                nc.vector.tensor_tensor(
                                    out=sel_t[:, :nj * P].rearrange(
                                        "p (n q) -> p n q", q=P),
                                    in0=iota_bc, in1=nid_bc,
                                    op=mybir.AluOpType.is_equal)
                            else:
                                for j in range(nj):
                                    nc.vector.tensor_scalar(
                                        out=sel_t[:, j * P:(j + 1) * P],
                                        in0=iota_row[:],
                                        scalar1=meta_t[:, moff + c0 + j:moff + c0 + j + 1],
                                        scalar2=1.0,
                                        op0=mybir.AluOpType.is_equal,
                                        op1=mybir.AluOpType.mult)
                        ps = pp.tile([P, D], mybir.dt.float32, tag="ps")
                        nc.tensor.matmul(
                            out=ps[:], lhsT=i_eye[:],
                            rhs=slab_t[:, soff + ti * D:soff + (ti + 1) * D],
                            start=True, stop=(nj == 0))
                        for j, gblk in enumerate(mm):
                            nc.tensor.matmul(
                                out=ps[:], lhsT=sel_t[:, j * P:(j + 1) * P],
                                rhs=G[:, gblk * D:(gblk + 1) * D],
                                start=False, stop=(j == nj - 1))
                        # PSUM holds K*mean; scale by 1/K on the drain
                        nc.scalar.activation(
                            out=obuf[:, ti * D:(ti + 1) * D], in_=ps[:],
                            func=mybir.ActivationFunctionType.Copy,
                            scale=float(plan.scale))
                    nc.sync.dma_start(
                        out=out[g["r0"]:g["r0"] + gt * P, :].rearrange(
                            "(b p) d -> p b d", p=P),
                        in_=obuf[:, :gt * D].rearrange(
                            "p (b d) -> p b d", d=D))
    nc.compile()
    return nc


# --------------------------------------------------------------------------
# v1 fallback: arbitrary sorted-or-not edge_seg


def build_program(n_tiles: int, K: int, weighted: bool,
                  g_bufs: int = 3, repeats: int = 1) -> bass.Bass:
    """Fallback program (per-column indirect DMA + tree reduction)."""
    nodes = n_tiles * P
    nc = bacc.Bacc("TRN2", target_bir_lowering=False)
    feat = nc.declare_dram_parameter("features", [N_TOTAL, D],
                                     mybir.dt.float32, isOutput=False)
    idx = nc.declare_dram_parameter("idx", [nodes, K],
                                    mybir.dt.int32, isOutput=False)
    if weighted:
        wts = nc.declare_dram_parameter("wts", [nodes, K],
                                        mybir.dt.float32, isOutput=False)
    out = nc.declare_dram_parameter("out", [nodes, D],
                                    mybir.dt.float32, isOutput=True)

    with tile.TileContext(nc) as tc:
        with tc.tile_pool(name="gath", bufs=g_bufs) as gp, \
             tc.tile_pool(name="io", bufs=4) as iop, \
             tc.tile_pool(name="res", bufs=4) as rp:
            for t in range(n_tiles * repeats):
                t = t % n_tiles
                sl = slice(t * P, (t + 1) * P)
                idx_t = iop.tile([P, K], mybir.dt.int32, tag="idx")
                nc.sync.dma_start(out=idx_t[:], in_=idx[sl, :])
                G = gp.tile([P, K * D], mybir.dt.float32, tag="g")
                for j in range(K):
                    nc.gpsimd.indirect_dma_start(
                        out=G[:, j * D:(j + 1) * D],
                        out_offset=None,
                        in_=feat[:],
                        in_offset=bass.IndirectOffsetOnAxis(
                            ap=idx_t[:, j:j + 1], axis=0),
                    )
                if weighted:
                    w_t = iop.tile([P, K], mybir.dt.float32, tag="w")
                    nc.sync.dma_start(out=w_t[:], in_=wts[sl, :])
                    for j in range(K):
                        nc.vector.tensor_scalar_mul(
                            out=G[:, j * D:(j + 1) * D],
                            in0=G[:, j * D:(j + 1) * D],
                            scalar1=w_t[:, j:j + 1],
                        )
                cur = K
                while cur > 1:
                    h = cur // 2
                    nc.vector.tensor_tensor(
                        out=G[:, :h * D],
                        in0=G[:, :h * D],
                        in1=G[:, h * D:2 * h * D],
                        op=mybir.AluOpType.add,
                    )
                    if cur % 2:
                        nc.vector.tensor_tensor(
                            out=G[:, (h - 1) * D:h * D],
                            in0=G[:, (h - 1) * D:h * D],
                            in1=G[:, (cur - 1) * D:cur * D],
                            op=mybir.AluOpType.add,
                        )
                    cur = h
                o_t = rp.tile([P, D], mybir.dt.float32, tag="o")
                if weighted:
                    nc.vector.tensor_copy(out=o_t[:], in_=G[:, :D])
                else:
                    nc.vector.tensor_scalar_mul(out=o_t[:], in0=G[:, :D],
                                                scalar1=1.0 / K)
                nc.sync.dma_start(out=out[sl, :], in_=o_t[:])
    nc.compile()
    return nc


def _prep_general(edge_seg, edge_dst, B):
    E = edge_dst.shape[0]
    order = np.argsort(edge_seg, kind="stable")
    sseg = edge_seg[order].astype(np.int64)
    sdst = edge_dst[order].astype(np.int32)
    counts = np.bincount(sseg, minlength=B).astype(np.int64)
    K = max(int(counts.max()), 1) if E else 1
    starts = np.zeros(B, np.int64)
    np.cumsum(counts[:-1], out=starts[1:])
    pos = np.arange(E, dtype=np.int64) - np.repeat(starts, counts)
    idx_mat = np.zeros((B, K), np.int32)
    wts_mat = np.zeros((B, K), np.float32)
    idx_mat[sseg, pos] = sdst
    inv = np.zeros(B, np.float32)
    nz = counts > 0
    inv[nz] = 1.0 / counts[nz]
    wts_mat[sseg, pos] = inv[sseg]
    return idx_mat, wts_mat, K


def kernel(features, edge_seg, edge_dst, num_nodes=None, **_unused):
    features = np.ascontiguousarray(np.asarray(features, dtype=np.float32))
    edge_seg = np.asarray(edge_seg)
    edge_dst = np.asarray(edge_dst)
    E = int(edge_dst.shape[0])
    if num_nodes is not None:
        B = int(np.asarray(num_nodes))
    else:
        B = int(edge_seg.max()) + 1

    K = E // B if B and E % B == 0 else 0
    uniform = K > 0 and np.array_equal(
        edge_seg, np.repeat(np.arange(B, dtype=edge_seg.dtype), K))
    if uniform:
        idx_mat = np.ascontiguousarray(edge_dst.reshape(B, K).astype(np.int32))
        wts_mat = None
    else:
        idx_mat, wts_mat, K = _prep_general(edge_seg, edge_dst, B)

    npc = -(-B // N_CORES)           # nodes per core (ceil)
    n_tiles = -(-npc // P)
    nodes_pad = n_tiles * P
    weighted = wts_mat is not None

    in_maps = None
    if not weighted:
        try:
            features_bf = features.astype(_bf16())
            in_maps, plans = prep_v3_core_maps(features_bf, idx_mat, B,
                                               npc, n_tiles)
            nc = build_program_v3(plans)
            for m in in_maps:
                m.pop("_plan", None)
        except (ValueError, OverflowError, AssertionError):
            in_maps = None

    if in_maps is None:
        nc = build_program(n_tiles, K, weighted)
        in_maps = []
        for c in range(N_CORES):
            lo = c * npc
            hi = min(B, (c + 1) * npc)
            idx_c = np.zeros((nodes_pad, K), np.int32)
            if hi > lo:
                idx_c[:hi - lo] = idx_mat[lo:hi]
            m = {"features": features, "idx": idx_c}
            if weighted:
                w_c = np.zeros((nodes_pad, K), np.float32)
                if hi > lo:
                    w_c[:hi - lo] = wts_mat[lo:hi]
                m["wts"] = w_c
            in_maps.append(m)

    kw = dict(TRACE_KWARGS) if TRACE else {}
    res = run_bass_kernel_spmd(nc, in_maps, list(range(N_CORES)), **kw)
    global LAST_RESULT
    LAST_RESULT = res
    parts = []
    for c in range(N_CORES):
        lo = c * npc
        hi = min(B, (c + 1) * npc)
        if hi > lo:
            parts.append(res.results[c]["out"][:hi - lo])
    return np.concatenate(parts, axis=0)


def prep_v3_core_maps(features_bf, idx_mat, B, npc, n_tiles):
    """Per-core host prep.  The program layout is data-dependent and differs
    per core, but SPMD needs ONE program: merge the per-core plans into a
    shared super-layout (max blocks per (group, window, tile) across cores)
    and re-emit each core's meta in that layout."""
    K = idx_mat.shape[1]
    if not np.array_equal(idx_mat[:, 0], np.arange(B, dtype=idx_mat.dtype)):
        raise ValueError("v3 requires a self-loop first column")
    Kg = K - 1
    nodes_pad = n_tiles * P
    bf = _bf16()
    wbounds = [w * WINDOW for w in range(N_WINDOWS + 1)]
    wbounds[-1] = N_TOTAL + 1

    # pass 1: per (core, tile, window) entries + exact block counts
    core_ent = []
    counts = np.zeros((N_CORES, n_tiles, N_WINDOWS), np.int64)
    sls = []
    for c in range(N_CORES):
        lo = c * npc
        hi = min(B, (c + 1) * npc)
        cols = np.full((nodes_pad, Kg), -1, np.int32)
        sl_c = np.zeros((nodes_pad, D), bf)
        if hi > lo:
            cols[:hi - lo] = idx_mat[lo:hi, 1:]
            n_sl = min(nodes_pad, N_TOTAL - lo)
            sl_c[:n_sl] = features_bf[lo:lo + n_sl]
        sls.append(sl_c)
        ent = {}
        for t in range(n_tiles):
            rf = cols[t * P:(t + 1) * P]
            r = rf.ravel().astype(np.int64)
            p = np.repeat(np.arange(P, dtype=np.int64), Kg)
            keep = r >= 0
            r, p = r[keep], p[keep]
            o = np.argsort(r, kind="stable")
            rs, ps = r[o], p[o]
            bounds = np.searchsorted(rs, wbounds)
            for w in range(N_WINDOWS):
                s0, s1 = bounds[w], bounds[w + 1]
                ent[(t, w)] = (rs[s0:s1] - w * WINDOW, ps[s0:s1])
                counts[c, t, w] = s1 - s0
        core_ent.append(ent)
    # shared layout: per-(tile, window) slot spans = max count across
    # cores.  Tiles pack contiguously inside each (group, window) region
    # (no per-tile block ceiling); a block straddling two tiles is simply
    # matmul'd by both with its own nid column each (entries of the other
    # tile carry nid -1 there).
    scnt = counts.max(axis=0)         # [n_tiles, N_WINDOWS] shared spans

    plan = _V3Plan()
    plan.n_tiles = n_tiles
    plan.scale = 1.0 / K
    metas = [[] for _ in range(N_CORES)]
    goff = 0
    tbg_max = 0
    meta_max = 0
    for g0 in range(0, n_tiles, GROUP_TILES):
        tiles = list(range(g0, min(g0 + GROUP_TILES, n_tiles)))
        gt = len(tiles)
        s0 = {}                       # (t, w) -> start slot in its region
        rblocks = []                  # blocks per window region
        for w in range(N_WINDOWS):
            s = 0
            for t in tiles:
                s0[(t, w)] = s
                span = int(scnt[t, w])
                if ALIGN_TILES:
                    span = -(-span // P) * P
                s += span
            rblocks.append(-(-s // P))
        breg = np.cumsum([0] + rblocks)   # block offset of each region
        tbg = int(breg[-1])
        tbg_max = max(tbg_max, tbg)
        # calls: split each window region at MAX_CALL_BLOCKS
        call_list = []                # (w, b0, nb, iw0)
        iw = 0
        for w in range(N_WINDOWS):
            cb = 0
            while cb < rblocks[w]:
                nb = min(MAX_CALL_BLOCKS, rblocks[w] - cb)
                call_list.append((w, int(breg[w]) + cb, nb, iw))
                iw += nb * 8
                cb += nb
        # matmul lists: tile t covers blocks [s0//P, ceil((s0+span)/P))
        tile_list = []
        col = 0
        nidcol_ix = {}
        for ti, t in enumerate(tiles):
            mm = []
            c0 = iw + col
            for w in range(N_WINDOWS):
                span = int(scnt[t, w])
                if not span:
                    continue
                j0 = s0[(t, w)] // P
                j1 = -(-(s0[(t, w)] + span) // P)
                for j in range(j0, j1):
                    mm.append(int(breg[w]) + j)
                    nidcol_ix[(t, w, j)] = col
                    col += 1
            tile_list.append((ti, mm, c0))
        ncols = col
        glen = iw + ncols
        # per-core meta piece
        for c in range(N_CORES):
            ent = core_ent[c]
            piece = np.zeros((128, glen), np.int16)
            for w in range(N_WINDOWS):
                idxarr = np.zeros(rblocks[w] * P, np.int16)
                nidarr = np.full(rblocks[w] * P, -1, np.int16)
                for t in tiles:
                    rel, ps = ent[(t, w)]
                    a = s0[(t, w)]
                    span = int(scnt[t, w])
                    if ALIGN_TILES:
                        span = -(-span // P) * P
                    idxarr[a:a + rel.shape[0]] = rel.astype(np.int16)
                    # pad slots re-read the last real row (row-buffer hit)
                    # instead of hammering row 0 of the window
                    if span > rel.shape[0]:
                        pad_idx = rel[-1] if rel.shape[0] else 0
                        idxarr[a + rel.shape[0]:a + span] = pad_idx
                    if a + span < idxarr.shape[0] and t == tiles[-1]:
                        # region tail (block rounding) after the last tile
                        idxarr[a + span:] = idxarr[a + span - 1] if span \
                            else 0
                    nidarr[a:a + ps.shape[0]] = ps.astype(np.int16)
                for (w2, b0, nb, iw0) in call_list:
                    if w2 != w:
                        continue
                    cb0 = b0 - int(breg[w])
                    e = idxarr[cb0 * P:(cb0 + nb) * P]
                    arr = e.reshape(nb * 8, 16).T
                    piece[:, iw0:iw0 + nb * 8] = np.tile(arr, (8, 1))
                # nid columns for this window, per tile (int16, one per sel)
                for ti, t in enumerate(tiles):
                    span = int(scnt[t, w])
                    if not span:
                        continue
                    a, b = s0[(t, w)], s0[(t, w)] + span
                    cnt_c = ent[(t, w)][1].shape[0]
                    j0 = a // P
                    j1 = -(-b // P)
                    for j in range(j0, j1):
                        colv = np.full(P, -1, np.int16)
                        lo_s = max(a, j * P)
                        hi_s = min(a + cnt_c, (j + 1) * P)
                        if hi_s > lo_s:
                            colv[lo_s - j * P:hi_s - j * P] = \
                                nidarr[lo_s:hi_s]
                        cix = nidcol_ix[(t, w, j)]
                        piece[:, iw + cix] = colv
            metas[c].append(piece)
        plan.groups.append(dict(r0=g0 * P, gt=gt, goff=goff, glen=glen,
                                tbg=tbg, calls=call_list, tiles=tile_list))
        goff += glen
        meta_max = max(meta_max, glen)
    plan.tbg_max = tbg_max
    plan.meta_max = meta_max
    plan.njmax = max(len(mm) for g in plan.groups for (_, mm, _) in g["tiles"])
    in_maps = []
    for c in range(N_CORES):
        meta_c = np.ascontiguousarray(np.concatenate(metas[c], axis=1))
        in_maps.append({"features": features_bf, "selfloop": sls[c],
                        "meta": meta_c})
    plan.meta = in_maps[0]["meta"]
    return in_maps, plan



# revision 21
# speedup vs baseline: 1.2366x; 1.0000x over previous
"""MeanAggregator (GNN segment-mean) Bass kernel for 8 Trainium2 NeuronCores.

Reference computation:
    gathered = features[edge_dst]                       # [E, D]
    sums     = segment_sum(gathered, edge_seg, B)       # [B, D]
    counts   = segment_sum(ones(E), edge_seg, B)        # [B]
    out      = sums / counts[:, None]                   # [B, D]

Strategy: shard output nodes (segments) contiguously across the 8 cores;
edge_seg is sorted, so each core owns its output rows outright -- no
collectives.

v4 fast path (uniform degree K with a self-loop column):
  * features are converted to bf16 host-side: halves the gather bytes and
    doubles the PE matmul rate; the 2e-2 tolerance leaves bf16's ~2e-3
    mean-of-17 error a wide margin.
  * neighbor rows are fetched with the GpSimd `dma_gather` ucode.  Indices
    are int16 relative to one of four 26624-row windows.  The gather is
    descriptor-rate bound (~35ns per 256B row per SDMA engine, independent
    of HBM locality -- measured via IDXSEQ/IDX0 ablations), so FEWER+larger
    calls win: 4 windows x 9 groups = 36 calls over 4 SWDGE queues (the
    ucode cap; each queue has a dedicated Q7 cpu pair for descgen).
  * the layout is computed from the actual indices at kernel() time: per
    (group of 6 tiles, window) the 8 cores' entries pack contiguously with
    shared per-tile spans (max count across cores, ~8% padding).  Pad
    slots re-read the preceding real row (row-buffer hit; an all-same-row
    pad serializes HBM banks -- IDX0 measured 2.4x slower).  Every slot is
    always written (no NaN hazard, no memsets).  A block straddling two
    tiles is matmul'd by both with its own nid column each.
  * the gathered rows land row-sorted, not per-node; a per-block [128,128]
    bf16 0/1 selection matrix routes them: out[node] += sel.T @ G_block on
    the PE, accumulating in PSUM.  ALL of a tile's sels build in ONE DVE
    tensor_tensor (iota is_equal nid), nid columns broadcast over the node
    axis via stride-0 APs; int16 operands keep DVE 2x mode.  This cut DVE
    busy ~2.6x vs per-sel tensor_scalar: DVE 2x-mode time serializes
    against SWDGE descgen (SBUF port conflict with the descriptor rings),
    so it trades ~1:1 against gather time.  GpSimd must NOT build sels.
  * the self-loop column is a contiguous slab (HWDGE line rate) folded in
    with an identity matmul; the 1/K mean scale is folded into the
    Activation-engine PSUM drain (activation Copy with scale), so sels
    stay exact 0/1.
  * all meta + self-loop slabs preload once at start (PRELOAD): the steady
    state has no HWDGE input DMAs contending with the gather rings; G is
    triple-buffered (G_BUFS=3) so all 4 queues stay fed across group
    boundaries.
  * engine budget per core: gather ~225us (the wall), PE ~1050 matmuls
    ~90-170us, DVE ~50us, ACT ~10us -- compute fully hides under the
    gather; total ~= gather + one group's compute tail.  Measured total
    ~225-240us (vs 339us baseline; shared-host timing variance is large --
    identical programs measured 1.5x apart minutes apart).

Fallback (v1, arbitrary counts): per-column indirect DMAs + VectorE tree
reduction with per-edge weights.
"""

import sys

for _p in ("/opt/trn_rl_repo", "/root/.axon_site/_ro/trn_rl_repo"):
    if _p not in sys.path:
        sys.path.append(_p)

import numpy as np

from concourse import bacc, bass, mybir
import concourse.tile as tile
from concourse.bass_utils import run_bass_kernel_spmd

TRACE = False            # set by test.py to profile the HW run
TRACE_KWARGS = {"trace": True}
LAST_RESULT = None

P = 128          # SBUF partitions = nodes per tile
D = 128          # feature dim
N_CORES = 8
N_TOTAL = 100000  # feature table rows

WINDOW = 26624           # rows per int16-index window (26624 = 208*128; 4
                         # windows minimizes dma_gather calls and padding --
                         # HW-tuned; int16 idx caps a window at 32767 rows)
N_WINDOWS = 4            # ceil(N_TOTAL / WINDOW)


def set_window(w):
    global WINDOW, N_WINDOWS
    WINDOW = w
    N_WINDOWS = -(-N_TOTAL // w)

G_BUFS = 3               # gather-buffer pipelining depth (3 keeps all 4
                         # SWDGE queues fed across group boundaries)
GROUP_TILES = 6          # tiles per gather group
MAX_CALL_BLOCKS = 42     # max 128-row blocks per dma_gather call
V3_QUEUES = 4            # SWDGE queues to round-robin gather calls over
SINGLE_PACKET = False    # True packs a call into one packet but caps it at
                         # 64 descs = 1024 idxs; False allows larger calls
SEL_PATTERN = "vvvvva"   # unused in v4 (kept for test.py compat)
BATCH_SEL = True         # one tensor_tensor builds ALL of a tile's sels
                         # (broadcast nid via stride-0 AP); False falls back
                         # to one tensor_scalar per sel on DVE
SEL_BUFS = 4             # sel tile lookahead (per-tile batches)
PRELOAD = True           # load all meta + self-loop slabs once up front
ALIGN_TILES = False      # True: block-align each tile's span (fewer matmuls,
                         # more gather rows); False: pack tight (fewer rows,
                         # boundary blocks matmul'd by both tiles)


def _scratch_size():
    """SWDGE ring must hold one call's descriptors (16 B/desc mirror of
    ucode); the ring is a power-of-two circular buffer."""
    # single_packet ucode packs 16 idxs/descriptor -> MCB*8 descs + slack
    need = 16 * (MAX_CALL_BLOCKS * 8 + 64)
    sz = 16384
    while sz < need:
        sz *= 2
    return sz


def _bf16():
    return mybir.dt.np(mybir.dt.bfloat16)


# --------------------------------------------------------------------------
# v3 host-side planning


class _V3Plan:
    """Static per-core program layout, derived from the actual indices."""

    __slots__ = ("groups", "meta", "tbg_max", "meta_max", "n_tiles", "scale",
                 "njmax")

    def __init__(self):
        self.groups = []


def build_program_v3(plan, repeats: int = 1, parts: str = "all") -> bass.Bass:
    """Bass program run identically on every core.

    Inputs per core:
      features [N_TOTAL, D] bf16  (replicated table)
      selfloop [n_tiles*P, D] bf16 (this core's self-loop rows, contiguous)
      meta     [128, meta_total] i16 (idx words + nid columns per group)
    Output per core:
      out      [n_tiles*P, D] f32

    parts: "all" (normal), "gather" (skip compute; timing ablation only),
    "compute" (skip gathers; broken on purpose, timing ablation only).
    """
    n_tiles = plan.n_tiles
    nodes = n_tiles * P
    bf16 = mybir.dt.bfloat16
    nc = bacc.Bacc("TRN2", target_bir_lowering=False,
                   num_swdge_queues=V3_QUEUES,
                   dynamic_dma_scratch_size=_scratch_size())
    feat = nc.declare_dram_parameter("features", [N_TOTAL, D],
                                     bf16, isOutput=False)
    sl = nc.declare_dram_parameter("selfloop", [nodes, D],
                                   bf16, isOutput=False)
    meta = nc.declare_dram_parameter("meta", [128, plan.meta.shape[1]],
                                     mybir.dt.int16, isOutput=False)
    # out is partition-major [P, n_tiles*D]: out[p, t*D+d] = mean[t*P+p, d].
    # Both DMA sides are then per-partition contiguous (128 big descriptors
    # instead of 768 small strided ones per group -- the small ones steal
    # SDMA engine time from the gather rings); the host unpacks.
    out = nc.declare_dram_parameter("out", [P, n_tiles * D],
                                    mybir.dt.float32, isOutput=True)
    gt_max = max(g["gt"] for g in plan.groups)

    njmax = max(1, plan.njmax)
    with tile.TileContext(nc) as tc:
        with tc.tile_pool(name="const", bufs=1) as cp, \
             tc.tile_pool(name="meta", bufs=3) as mp, \
             tc.tile_pool(name="slab", bufs=3) as slp, \
             tc.tile_pool(name="gath", bufs=G_BUFS) as gp, \
             tc.tile_pool(name="sel", bufs=SEL_BUFS) as sp_, \
             tc.tile_pool(name="ps", bufs=4, space="PSUM") as pp, \
             tc.tile_pool(name="res", bufs=3) as rp:
            # iota_row[p, q] = q ; piota[p, q] = p      (int16)
            iota_row = cp.tile([P, P], mybir.dt.int16)
            nc.gpsimd.iota(iota_row[:], pattern=[[1, P]], channel_multiplier=0)
            piota_i = cp.tile([P, P], mybir.dt.int32)
            nc.gpsimd.iota(piota_i[:], pattern=[[0, P]],
                           channel_multiplier=1)
            piota = cp.tile([P, P], mybir.dt.float32)
            nc.vector.tensor_copy(out=piota[:], in_=piota_i[:])
            # iota_b[p, (j, q)] = q  (int16): batched sel-build operand
            iota_b = cp.tile([P, njmax * P], mybir.dt.int16)
            nc.gpsimd.iota(iota_b[:], pattern=[[0, njmax], [1, P]],
                           channel_multiplier=0)
            # I[p, q] = (q == p)                        (bf16; 1/K folded
            # into the PSUM drain activation)
            i_eye = cp.tile([P, P], bf16)
            nc.vector.tensor_scalar(
                out=i_eye[:], in0=iota_row[:],
                scalar1=piota[:, 0:1], scalar2=1.0,
                op0=mybir.AluOpType.is_equal, op1=mybir.AluOpType.mult)
            if PRELOAD:
                # load ALL meta + self-loop slabs once up front: removes
                # per-group HWDGE input DMAs from the steady state (they
                # contend with the gather rings on the SDMA engines)
                meta_all = cp.tile([128, plan.meta.shape[1]], mybir.dt.int16)
                nc.sync.dma_start(out=meta_all[:], in_=meta[:, :])
                slab_all = cp.tile([P, n_tiles * D], bf16)
                nc.scalar.dma_start(
                    out=slab_all[:].rearrange("p (b d) -> p b d", d=D),
                    in_=sl[:, :].rearrange("(b p) d -> p b d", p=P))
            for rep in range(repeats):
                for g in plan.groups:
                    gt, tbg = g["gt"], g["tbg"]
                    if PRELOAD:
                        meta_t, moff = meta_all, g["goff"]
                        slab_t, soff = slab_all, (g["r0"] // P) * D
                    else:
                        moff = soff = 0
                        meta_t = mp.tile([128, plan.meta_max],
                                         mybir.dt.int16, tag="meta")
                        nc.sync.dma_start(
                            out=meta_t[:, :g["glen"]],
                            in_=meta[:, g["goff"]:g["goff"] + g["glen"]])
                        slab_t = slp.tile([P, gt_max * D], bf16, tag="slab")
                        nc.scalar.dma_start(
                            out=slab_t[:, :gt * D].rearrange(
                                "p (b d) -> p b d", d=D),
                            in_=sl[g["r0"]:g["r0"] + gt * P, :].rearrange(
                                "(b p) d -> p b d", p=P))
                    G = gp.tile([P, plan.tbg_max * D], bf16, tag="g")
                    if parts == "compute":
                        g_calls = []
                        # keep compute numerics finite without gathers
                        nc.vector.memset(G[:], 0.0)
                    else:
                        g_calls = g["calls"]
                    gi = g["r0"] // (P * GROUP_TILES)
                    for c, (w, b0, nb, iw0) in enumerate(g_calls):
                        wbase = w * WINDOW
                        wsize = min(WINDOW, N_TOTAL - wbase)
                        nc.gpsimd.dma_gather(
                            out_ap=G[:, b0 * D:(b0 + nb) * D].rearrange(
                                "p (b d) -> p b d", d=D),
                            in_ap=feat[wbase:wbase + wsize],
                            idxs_ap=meta_t[:, moff + iw0:moff + iw0 + nb * 8],
                            num_idxs=nb * P,
                            num_idxs_reg=nb * P,
                            elem_size=D,
                            single_packet=SINGLE_PACKET,
                            queue_num=(c + gi) % V3_QUEUES,
                        )
                    obuf = rp.tile([P, gt_max * D], mybir.dt.float32,
                                   tag="o")
                    if parts == "gather":
                        # touch G so the gather isn't dead-code'd, then skip
                        # the compute stage
                        dummy = rp.tile([P, D], mybir.dt.float32, tag="dm")
                        nc.vector.tensor_copy(out=dummy[:],
                                              in_=G[:, :2 * D].bitcast(
                                                  mybir.dt.float32))
                        goff2 = (g["r0"] // P) * D
                        nc.sync.dma_start(
                            out=out[:, goff2:goff2 + D], in_=dummy[:])
                        continue
                    for (ti, mm, c0) in g["tiles"]:
                        nj = len(mm)
                        sel_t = sp_.tile([P, njmax * P], bf16, tag="sel")
                        if nj:
                            if BATCH_SEL:
                                # sel_t[p, (j, q)] = (q == nid_j[p]) in ONE
                                # DVE op: nid column j broadcast over q via
                                # stride-0 AP
                                iota3 = iota_b[:, :nj * P].rearrange(
                                    "p (n q) -> p n q", q=P)
                                nid3 = meta_t[:, moff + c0:moff + c0 + nj].rearrange(
                                    "p (n o) -> p n o", o=1)
                                nid_bc, iota_bc = bass.broadcast_tensor_aps(
                                    nid3, iota3)
                                nc.vector.tensor_tensor(
                                    out=sel_t[:, :nj * P].rearrange(
                                        "p (n q) -> p n q", q=P),
                                    in0=iota_bc, in1=nid_bc,
                                    op=mybir.AluOpType.is_equal)
                            else:
                                for j in range(nj):
                                    nc.vector.tensor_scalar(
                                        out=sel_t[:, j * P:(j + 1) * P],
                                        in0=iota_row[:],
                                        scalar1=meta_t[:, moff + c0 + j:moff + c0 + j + 1],
                                        scalar2=1.0,
                                        op0=mybir.AluOpType.is_equal,
                                        op1=mybir.AluOpType.mult)
                        ps = pp.tile([P, D], mybir.dt.float32, tag="ps")
                        nc.tensor.matmul(
                            out=ps[:], lhsT=i_eye[:],
                            rhs=slab_t[:, soff + ti * D:soff + (ti + 1) * D],
                            start=True, stop=(nj == 0))
                        for j, gblk in enumerate(mm):
                            nc.tensor.matmul(
                                out=ps[:], lhsT=sel_t[:, j * P:(j + 1) * P],
                                rhs=G[:, gblk * D:(gblk + 1) * D],
                                start=False, stop=(j == nj - 1))
                        # PSUM holds K*mean; scale by 1/K on the drain
                        nc.scalar.activation(
                            out=obuf[:, ti * D:(ti + 1) * D], in_=ps[:],
                            func=mybir.ActivationFunctionType.Copy,
                            scale=float(plan.scale))
                    goff2 = (g["r0"] // P) * D
                    nc.sync.dma_start(
                        out=out[:, goff2:goff2 + gt * D],
                        in_=obuf[:, :gt * D])
    nc.compile()
    return nc


# --------------------------------------------------------------------------
# v1 fallback: arbitrary sorted-or-not edge_seg


def build_program(n_tiles: int, K: int, weighted: bool,
                  g_bufs: int = 3, repeats: int = 1) -> bass.Bass:
    """Fallback program (per-column indirect DMA + tree reduction)."""
    nodes = n_tiles * P
    nc = bacc.Bacc("TRN2", target_bir_lowering=False)
    feat = nc.declare_dram_parameter("features", [N_TOTAL, D],
                                     mybir.dt.float32, isOutput=False)
    idx = nc.declare_dram_parameter("idx", [nodes, K],
                                    mybir.dt.int32, isOutput=False)
    if weighted:
        wts = nc.declare_dram_parameter("wts", [nodes, K],
                                        mybir.dt.float32, isOutput=False)
    out = nc.declare_dram_parameter("out", [nodes, D],
                                    mybir.dt.float32, isOutput=True)

    with tile.TileContext(nc) as tc:
        with tc.tile_pool(name="gath", bufs=g_bufs) as gp, \
             tc.tile_pool(name="io", bufs=4) as iop, \
             tc.tile_pool(name="res", bufs=4) as rp:
            for t in range(n_tiles * repeats):
                t = t % n_tiles
                sl = slice(t * P, (t + 1) * P)
                idx_t = iop.tile([P, K], mybir.dt.int32, tag="idx")
                nc.sync.dma_start(out=idx_t[:], in_=idx[sl, :])
                G = gp.tile([P, K * D], mybir.dt.float32, tag="g")
                for j in range(K):
                    nc.gpsimd.indirect_dma_start(
                        out=G[:, j * D:(j + 1) * D],
                        out_offset=None,
                        in_=feat[:],
                        in_offset=bass.IndirectOffsetOnAxis(
                            ap=idx_t[:, j:j + 1], axis=0),
                    )
                if weighted:
                    w_t = iop.tile([P, K], mybir.dt.float32, tag="w")
                    nc.sync.dma_start(out=w_t[:], in_=wts[sl, :])
                    for j in range(K):
                        nc.vector.tensor_scalar_mul(
                            out=G[:, j * D:(j + 1) * D],
                            in0=G[:, j * D:(j + 1) * D],
                            scalar1=w_t[:, j:j + 1],
                        )
                cur = K
                while cur > 1:
                    h = cur // 2
                    nc.vector.tensor_tensor(
                        out=G[:, :h * D],
                        in0=G[:, :h * D],
                        in1=G[:, h * D:2 * h * D],
                        op=mybir.AluOpType.add,
                    )
                    if cur % 2:
                        nc.vector.tensor_tensor(
                            out=G[:, (h - 1) * D:h * D],
                            in0=G[:, (h - 1) * D:h * D],
                            in1=G[:, (cur - 1) * D:cur * D],
                            op=mybir.AluOpType.add,
                        )
                    cur = h
                o_t = rp.tile([P, D], mybir.dt.float32, tag="o")
                if weighted:
                    nc.vector.tensor_copy(out=o_t[:], in_=G[:, :D])
                else:
                    nc.vector.tensor_scalar_mul(out=o_t[:], in0=G[:, :D],
                                                scalar1=1.0 / K)
                nc.sync.dma_start(out=out[sl, :], in_=o_t[:])
    nc.compile()
    return nc


def _prep_general(edge_seg, edge_dst, B):
    E = edge_dst.shape[0]
    order = np.argsort(edge_seg, kind="stable")
    sseg = edge_seg[order].astype(np.int64)
    sdst = edge_dst[order].astype(np.int32)
    counts = np.bincount(sseg, minlength=B).astype(np.int64)
    K = max(int(counts.max()), 1) if E else 1
    starts = np.zeros(B, np.int64)
    np.cumsum(counts[:-1], out=starts[1:])
    pos = np.arange(E, dtype=np.int64) - np.repeat(starts, counts)
    idx_mat = np.zeros((B, K), np.int32)
    wts_mat = np.zeros((B, K), np.float32)
    idx_mat[sseg, pos] = sdst
    inv = np.zeros(B, np.float32)
    nz = counts > 0
    inv[nz] = 1.0 / counts[nz]
    wts_mat[sseg, pos] = inv[sseg]
    return idx_mat, wts_mat, K


def kernel(features, edge_seg, edge_dst, num_nodes=None, **_unused):
    features = np.ascontiguousarray(np.asarray(features, dtype=np.float32))
    edge_seg = np.asarray(edge_seg)
    edge_dst = np.asarray(edge_dst)
    E = int(edge_dst.shape[0])
    if num_nodes is not None:
        B = int(np.asarray(num_nodes))
    else:
        B = int(edge_seg.max()) + 1

    K = E // B if B and E % B == 0 else 0
    uniform = K > 0 and np.array_equal(
        edge_seg, np.repeat(np.arange(B, dtype=edge_seg.dtype), K))
    if uniform:
        idx_mat = np.ascontiguousarray(edge_dst.reshape(B, K).astype(np.int32))
        wts_mat = None
    else:
        idx_mat, wts_mat, K = _prep_general(edge_seg, edge_dst, B)

    npc = -(-B // N_CORES)           # nodes per core (ceil)
    n_tiles = -(-npc // P)
    nodes_pad = n_tiles * P
    weighted = wts_mat is not None

    in_maps = None
    if not weighted:
        try:
            features_bf = features.astype(_bf16())
            in_maps, plans = prep_v3_core_maps(features_bf, idx_mat, B,
                                               npc, n_tiles)
            nc = build_program_v3(plans)
            for m in in_maps:
                m.pop("_plan", None)
        except (ValueError, OverflowError, AssertionError):
            in_maps = None

    if in_maps is None:
        nc = build_program(n_tiles, K, weighted)
        in_maps = []
        for c in range(N_CORES):
            lo = c * npc
            hi = min(B, (c + 1) * npc)
            idx_c = np.zeros((nodes_pad, K), np.int32)
            if hi > lo:
                idx_c[:hi - lo] = idx_mat[lo:hi]
            m = {"features": features, "idx": idx_c}
            if weighted:
                w_c = np.zeros((nodes_pad, K), np.float32)
                if hi > lo:
                    w_c[:hi - lo] = wts_mat[lo:hi]
                m["wts"] = w_c
            in_maps.append(m)

    kw = dict(TRACE_KWARGS) if TRACE else {}
    res = run_bass_kernel_spmd(nc, in_maps, list(range(N_CORES)), **kw)
    global LAST_RESULT
    LAST_RESULT = res
    v3 = in_maps is not None and "meta" in in_maps[0]
    parts = []
    for c in range(N_CORES):
        lo = c * npc
        hi = min(B, (c + 1) * npc)
        if hi <= lo:
            continue
        o = res.results[c]["out"]
        if v3:
            # v3 out is partition-major [P, n_tiles*D]
            o = o.reshape(P, n_tiles, D).swapaxes(0, 1).reshape(-1, D)
        parts.append(o[:hi - lo])
    return np.concatenate(parts, axis=0)


def prep_v3_core_maps(features_bf, idx_mat, B, npc, n_tiles):
    """Per-core host prep.  The program layout is data-dependent and differs
    per core, but SPMD needs ONE program: merge the per-core plans into a
    shared super-layout (max blocks per (group, window, tile) across cores)
    and re-emit each core's meta in that layout."""
    K = idx_mat.shape[1]
    if not np.array_equal(idx_mat[:, 0], np.arange(B, dtype=idx_mat.dtype)):
        raise ValueError("v3 requires a self-loop first column")
    Kg = K - 1
    nodes_pad = n_tiles * P
    bf = _bf16()
    wbounds = [w * WINDOW for w in range(N_WINDOWS + 1)]
    wbounds[-1] = N_TOTAL + 1

    # pass 1: per (core, tile, window) entries + exact block counts
    core_ent = []
    counts = np.zeros((N_CORES, n_tiles, N_WINDOWS), np.int64)
    sls = []
    for c in range(N_CORES):
        lo = c * npc
        hi = min(B, (c + 1) * npc)
        cols = np.full((nodes_pad, Kg), -1, np.int32)
        sl_c = np.zeros((nodes_pad, D), bf)
        if hi > lo:
            cols[:hi - lo] = idx_mat[lo:hi, 1:]
            n_sl = min(nodes_pad, N_TOTAL - lo)
            sl_c[:n_sl] = features_bf[lo:lo + n_sl]
        sls.append(sl_c)
        ent = {}
        for t in range(n_tiles):
            rf = cols[t * P:(t + 1) * P]
            r = rf.ravel().astype(np.int64)
            p = np.repeat(np.arange(P, dtype=np.int64), Kg)
            keep = r >= 0
            r, p = r[keep], p[keep]
            o = np.argsort(r, kind="stable")
            rs, ps = r[o], p[o]
            bounds = np.searchsorted(rs, wbounds)
            for w in range(N_WINDOWS):
                s0, s1 = bounds[w], bounds[w + 1]
                ent[(t, w)] = (rs[s0:s1] - w * WINDOW, ps[s0:s1])
                counts[c, t, w] = s1 - s0
        core_ent.append(ent)
    # shared layout: per-(tile, window) slot spans = max count across
    # cores.  Tiles pack contiguously inside each (group, window) region
    # (no per-tile block ceiling); a block straddling two tiles is simply
    # matmul'd by both with its own nid column each (entries of the other
    # tile carry nid -1 there).
    scnt = counts.max(axis=0)         # [n_tiles, N_WINDOWS] shared spans

    plan = _V3Plan()
    plan.n_tiles = n_tiles
    plan.scale = 1.0 / K
    metas = [[] for _ in range(N_CORES)]
    goff = 0
    tbg_max = 0
    meta_max = 0
    for g0 in range(0, n_tiles, GROUP_TILES):
        tiles = list(range(g0, min(g0 + GROUP_TILES, n_tiles)))
        gt = len(tiles)
        s0 = {}                       # (t, w) -> start slot in its region
        rblocks = []                  # blocks per window region
        for w in range(N_WINDOWS):
            s = 0
            for t in tiles:
                s0[(t, w)] = s
                span = int(scnt[t, w])
                if ALIGN_TILES:
                    span = -(-span // P) * P
                s += span
            rblocks.append(-(-s // P))
        breg = np.cumsum([0] + rblocks)   # block offset of each region
        tbg = int(breg[-1])
        tbg_max = max(tbg_max, tbg)
        # calls: split each window region at MAX_CALL_BLOCKS
        call_list = []                # (w, b0, nb, iw0)
        iw = 0
        for w in range(N_WINDOWS):
            cb = 0
            while cb < rblocks[w]:
                nb = min(MAX_CALL_BLOCKS, rblocks[w] - cb)
                call_list.append((w, int(breg[w]) + cb, nb, iw))
                iw += nb * 8
                cb += nb
        # matmul lists: tile t covers blocks [s0//P, ceil((s0+span)/P))
        tile_list = []
        col = 0
        nidcol_ix = {}
        for ti, t in enumerate(tiles):
            mm = []
            c0 = iw + col
            for w in range(N_WINDOWS):
                span = int(scnt[t, w])
                if not span:
                    continue
                j0 = s0[(t, w)] // P
                j1 = -(-(s0[(t, w)] + span) // P)
                for j in range(j0, j1):
                    mm.append(int(breg[w]) + j)
                    nidcol_ix[(t, w, j)] = col
                    col += 1
            tile_list.append((ti, mm, c0))
        ncols = col
        glen = iw + ncols
        # per-core meta piece
        for c in range(N_CORES):
            ent = core_ent[c]
            piece = np.zeros((128, glen), np.int16)
            for w in range(N_WINDOWS):
                idxarr = np.zeros(rblocks[w] * P, np.int16)
                nidarr = np.full(rblocks[w] * P, -1, np.int16)
                for t in tiles:
                    rel, ps = ent[(t, w)]
                    a = s0[(t, w)]
                    span = int(scnt[t, w])
                    if ALIGN_TILES:
                        span = -(-span // P) * P
                    idxarr[a:a + rel.shape[0]] = rel.astype(np.int16)
                    # pad slots re-read the last real row (row-buffer hit)
                    # instead of hammering row 0 of the window
                    if span > rel.shape[0]:
                        pad_idx = rel[-1] if rel.shape[0] else 0
                        idxarr[a + rel.shape[0]:a + span] = pad_idx
                    if a + span < idxarr.shape[0] and t == tiles[-1]:
                        # region tail (block rounding) after the last tile
                        idxarr[a + span:] = idxarr[a + span - 1] if span \
                            else 0
                    nidarr[a:a + ps.shape[0]] = ps.astype(np.int16)
                for (w2, b0, nb, iw0) in call_list:
                    if w2 != w:
                        continue
                    cb0 = b0 - int(breg[w])
                    e = idxarr[cb0 * P:(cb0 + nb) * P]
                    arr = e.reshape(nb * 8, 16).T
                    piece[:, iw0:iw0 + nb * 8] = np.tile(arr, (8, 1))
                # nid columns for this window, per tile (int16, one per sel)
                for ti, t in enumerate(tiles):
                    span = int(scnt[t, w])
                    if not span:
                        continue
                    a, b = s0[(t, w)], s0[(t, w)] + span
                    cnt_c = ent[(t, w)][1].shape[0]
                    j0 = a // P
                    j1 = -(-b // P)
                    for j in range(j0, j1):
                        colv = np.full(P, -1, np.int16)
                        lo_s = max(a, j * P)
                        hi_s = min(a + cnt_c, (j + 1) * P)
                        if hi_s > lo_s:
                            colv[lo_s - j * P:hi_s - j * P] = \
                                nidarr[lo_s:hi_s]
                        cix = nidcol_ix[(t, w, j)]
                        piece[:, iw + cix] = colv
            metas[c].append(piece)
        plan.groups.append(dict(r0=g0 * P, gt=gt, goff=goff, glen=glen,
                                tbg=tbg, calls=call_list, tiles=tile_list))
        goff += glen
        meta_max = max(meta_max, glen)
    plan.tbg_max = tbg_max
    plan.meta_max = meta_max
    plan.njmax = max(len(mm) for g in plan.groups for (_, mm, _) in g["tiles"])
    in_maps = []
    for c in range(N_CORES):
        meta_c = np.ascontiguousarray(np.concatenate(metas[c], axis=1))
        in_maps.append({"features": features_bf, "selfloop": sls[c],
                        "meta": meta_c})
    plan.meta = in_maps[0]["meta"]
    return in_maps, plan

